# revision 1
# baseline (speedup 1.0000x reference)
"""Trainium2 Bass kernel for nn_MultiHeadAttention (B=4, S=2048, E=1024, H=16, D=64).

Sharding: 8 cores = 4 batches x 2 head-halves. Core c handles batch c//2 and
heads [ (c%2)*8, (c%2)*8+8 ). Each core computes its heads' attention and a
partial output projection; the host sums the two per-batch partials and adds bo.

Device-side dataflow (everything "transposed" so the contraction dim always
lands on SBUF partitions):
  qT/kT/vT [E, S] host-pretransposed activations.
  Q^T/K^T [d, s] tiles from the projections; the per-core 512 d-dims are
  host-permuted per 128-tile as [headA evens | headB evens | headA odds |
  headB odds], so RoPE pairs sit at partition distance 64 (two VectorE copies
  build the swapped operand straight from PSUM) and each head is two 32-row
  groups -> the scores matmuls for two heads pack into all four PE row groups
  concurrently (K=32 accumulating pairs).
  scores^T[k, q] with exp fused into the PSUM eviction on ScalarE over
  [128, 2x512] kt-pairs (scale=1/8; scores are small, no max needed).
  attn_out^T[d, q] = V_aug.T @ P^T with V_aug = [V | ones]: row 64 is the
  softmax denominator. Normalize via VectorE reciprocal + GpSimd partition
  broadcast. Output projection accumulates over d tiles and DMAs PSUM->DRAM.
"""

import os
import sys
import numpy as np

sys.path.insert(0, "/opt/trn_rl_repo")

from contextlib import ExitStack

import concourse.bacc as bacc
import concourse.tile as tile
from concourse import mybir
from concourse.bass_utils import run_bass_kernel_spmd

B, S, E = 4, 2048, 1024
H, D = 16, 64
HPC = 8          # heads per core
DPC = HPC * D    # 512 d-dims per core
P = 128
NSC = S // 512   # 4 s-chunks of 512
NST = S // 128   # 16 s-tiles of 128
NET = E // 128   # 8 e-tiles of 128
NDT = DPC // 128  # 4 d-tiles of 128

F32 = mybir.dt.float32
BF16 = mybir.dt.bfloat16

# dtype knob for matmul inputs: "bf16" | "f32" | "f32r"
MM_DTYPE = os.environ.get("MM_DTYPE", "bf16")
REPEAT = int(os.environ.get("KERNEL_REPEAT", "1"))
SPLIT_EXP = os.environ.get("SPLIT_EXP", "0") == "1"
SIMPLE_VAUG = os.environ.get("SIMPLE_VAUG", "0") == "1"
PACK = os.environ.get("PACK", "2")  # "2" = two K=64 MMs packed; "0" = v1 unpacked
PHASES = set(os.environ.get("PHASES", "qk,v,scores,attnv,final").split(","))
SCORES_KTP = int(os.environ.get("SCORES_KTP", str(NST // 2)))
ATTNV_KT = int(os.environ.get("ATTNV_KT", str(NST)))
ACT_OUT = os.environ.get("ACT_OUT", "0") == "1"


_IN_DT = BF16 if MM_DTYPE == "bf16" else F32


def _np_dt():
    if MM_DTYPE == "bf16":
        import ml_dtypes

        return np.dtype(ml_dtypes.bfloat16)
    return np.dtype(np.float32)


def _mm(ap):
    if MM_DTYPE == "f32r":
        return ap.bitcast(mybir.dt.float32r)
    return ap


def build_program(repeat=None):
    global REPEAT
    if repeat is not None:
        REPEAT = repeat
    nc = bacc.Bacc("TRN2", target_bir_lowering=False, debug=False, num_devices=8)

    dt_in = _IN_DT
    qT = nc.dram_tensor("qT", [E, S], dt_in, kind="ExternalInput").ap()
    kT = nc.dram_tensor("kT", [E, S], dt_in, kind="ExternalInput").ap()
    vT = nc.dram_tensor("vT", [E, S], dt_in, kind="ExternalInput").ap()
    wqT = nc.dram_tensor("wqT", [E, DPC], dt_in, kind="ExternalInput").ap()
    wkT = nc.dram_tensor("wkT", [E, DPC], dt_in, kind="ExternalInput").ap()
    wvT = nc.dram_tensor("wvT", [E, DPC], dt_in, kind="ExternalInput").ap()
    woT = nc.dram_tensor("woT", [DPC, E], dt_in, kind="ExternalInput").ap()
    ctab = nc.dram_tensor("ctab", [P, S], F32, kind="ExternalInput").ap()
    stab = nc.dram_tensor("stab", [P, S], F32, kind="ExternalInput").ap()
    out = nc.dram_tensor("out", [S, E], F32, kind="ExternalOutput").ap()

    with tile.TileContext(nc) as tc:
        with ExitStack() as ctx:
            body(ctx, tc, nc, qT, kT, vT, wqT, wkT, wvT, woT, ctab, stab, out)
    nc.compile()
    return nc


def body(ctx, tc, nc, qT, kT, vT, wqT, wkT, wvT, woT, ctab, stab, out):
    dt_in = _IN_DT

    consts = ctx.enter_context(tc.tile_pool(name="consts", bufs=1))
    c_sb = consts.tile([P, S], F32, tag="ctab")
    s_sb = consts.tile([P, S], F32, tag="stab")
    nc.sync.dma_start(out=c_sb[:], in_=ctab[:])
    nc.sync.dma_start(out=s_sb[:], in_=stab[:])

    wpool = ctx.enter_context(tc.tile_pool(name="wpool", bufs=16))
    wopool = ctx.enter_context(tc.tile_pool(name="wopool", bufs=4))
    # full e-row tiles [128, S] of qT/kT/vT, reused across the three phases
    xpool = ctx.enter_context(tc.tile_pool(name="xpool", bufs=16))

    qkrot = ctx.enter_context(tc.tile_pool(name="qkrot", bufs=12))
    vaug_pool = ctx.enter_context(tc.tile_pool(name="vaug", bufs=1))
    aall_pool = ctx.enter_context(tc.tile_pool(name="aall", bufs=4))
    pt_pool = ctx.enter_context(tc.tile_pool(name="pt", bufs=2))

    rtmp = ctx.enter_context(tc.tile_pool(name="rtmp", bufs=2))
    stg = ctx.enter_context(tc.tile_pool(name="stg", bufs=4))
    ntmp = ctx.enter_context(tc.tile_pool(name="ntmp", bufs=2))

    opool = ctx.enter_context(tc.tile_pool(name="opool", bufs=2))
    psum_a = ctx.enter_context(tc.tile_pool(name="psum_a", bufs=3, space="PSUM"))
    psum_b = ctx.enter_context(tc.tile_pool(name="psum_b", bufs=2, space="PSUM"))

    pools = (c_sb, s_sb, wpool, wopool, xpool, qkrot, vaug_pool, aall_pool,
             pt_pool, rtmp, stg, ntmp, opool, psum_a, psum_b)
    for rep in range(REPEAT):
        one_pass(tc, nc, qT, kT, vT, wqT, wkT, wvT, woT, out, *pools)


def one_pass(tc, nc, qT, kT, vT, wqT, wkT, wvT, woT, out,
             c_sb, s_sb, wpool, wopool, xpool, qkrot, vaug_pool, aall_pool,
             pt_pool, rtmp, stg, ntmp, opool, psum_a, psum_b):
    dt_in = _IN_DT

    # ---------------- Q^T / K^T projections with fused RoPE ----------------
    # Q is stored per-head with the other head's 64 rows zeroed, so the scores
    # matmuls can contract over the full 128 partitions (K=128 enables fast
    # weight load; the zero rows contribute nothing).
    qrot = [qkrot.tile([P, S], dt_in, tag="qkrot", name=f"qz{i}") for i in range(2 * NDT)]
    krot = [qkrot.tile([P, S], dt_in, tag="qkrot", name=f"krot{i}") for i in range(NDT)]
    for t in range(NDT):
        nc.gpsimd.memset(qrot[2 * t][64:128, :], 0.0)
        nc.gpsimd.memset(qrot[2 * t + 1][0:64, :], 0.0)

    for (wT, rot_tiles, nm) in (((wqT, qrot, "q"), (wkT, krot, "k")) if "qk" in PHASES else ()):
        src = qT if nm == "q" else kT
        w_sb = [wpool.tile([P, DPC], dt_in, tag="w", name=f"w_{nm}{i}") for i in range(NET)]
        for et in range(NET):
            nc.sync.dma_start(out=w_sb[et][:], in_=wT[et * P:(et + 1) * P, :])
        for sc in range(NSC):
            ssl = slice(sc * 512, (sc + 1) * 512)
            x_sb = [xpool.tile([P, 512], dt_in, tag="x", name=f"x_{nm}{sc}_{i}")
                    for i in range(NET)]
            for et in range(NET):
                nc.sync.dma_start(out=x_sb[et][:], in_=src[et * P:(et + 1) * P, ssl])
            for t in range(NDT):
                ps = psum_a.tile([P, 2, 512], F32, tag="ps", name=f"ps_{nm}{sc}_{t}")
                for et in range(NET):
                    nc.tensor.matmul(
                        ps[:, 0, :], _mm(w_sb[et][:, t * P:(t + 1) * P]),
                        _mm(x_sb[et][:]),
                        start=(et == 0), stop=(et == NET - 1),
                    )
                # RoPE eviction (pairs at partition distance 64):
                #   rot = ps * C + swap64(ps) * Ssigned
                xsw = rtmp.tile([P, 512], F32, tag="xsw")
                for blk in range(4):
                    sb = blk ^ 1
                    nc.vector.tensor_copy(xsw[blk * 32:(blk + 1) * 32, :],
                                          ps[sb * 32:(sb + 1) * 32, 0, :])
                nc.vector.tensor_mul(xsw[:], xsw[:], s_sb[:, ssl])
                t2 = rtmp.tile([P, 512], F32, tag="t2")
                nc.vector.tensor_mul(t2[:], ps[:, 0, :], c_sb[:, ssl])
                if nm == "q":
                    nc.vector.tensor_add(rot_tiles[2 * t][0:64, ssl],
                                         t2[0:64, :], xsw[0:64, :])
                    nc.vector.tensor_add(rot_tiles[2 * t + 1][64:128, ssl],
                                         t2[64:128, :], xsw[64:128, :])
                else:
                    nc.vector.tensor_add(rot_tiles[t][:, ssl], t2[:], xsw[:])

    # ---------------- V projection -> V_aug with ones columns ----------------
    vaug = vaug_pool.tile([P, NST, HPC * 65], dt_in, tag="vaug")
    nc.vector.memset(vaug[:], 1.0)
    do_v = "v" in PHASES
    wv_sb = [wpool.tile([P, DPC], dt_in, tag="w", name=f"w_v{i}") for i in range(NET)]
    if do_v:
        for et in range(NET):
            nc.sync.dma_start(out=wv_sb[et][:], in_=wvT[et * P:(et + 1) * P, :])
    vaug_v = vaug.rearrange("p st (h dd) -> p st h dd", h=HPC)
    for sc in range(NSC if do_v else 0):
        xv_sb = [xpool.tile([P, 512], dt_in, tag="x", name=f"x_v{sc}_{i}")
                 for i in range(NET)]
        for et in range(NET):
            nc.sync.dma_start(out=xv_sb[et][:],
                              in_=vT[et * P:(et + 1) * P, sc * 512:(sc + 1) * 512])
        for sti in range(4):
            st = sc * 4 + sti
            ps = psum_a.tile([P, 2, 512], F32, tag="ps", name=f"ps_v{st}")
            for et in range(NET):
                nc.tensor.matmul(ps[:, 0, :],
                                 _mm(xv_sb[et][:, sti * P:(sti + 1) * P]),
                                 _mm(wv_sb[et][:]), start=(et == 0), stop=(et == NET - 1))
            nc.vector.tensor_copy(
                vaug_v[:, st, :, 0:64],
                ps[:, 0, :].rearrange("p (h d) -> p h d", h=HPC),
            )

    # ---------------- attention + output projection, per q-chunk ----------------
    aall = [aall_pool.tile([P, S], dt_in, tag="aall", name=f"aall{i}") for i in range(NDT)]
    wo_sb = [wopool.tile([P, E], dt_in, tag="wo", name=f"wo{i}") for i in range(NDT)]
    for t in range(NDT):
        nc.sync.dma_start(out=wo_sb[t][:], in_=woT[t * P:(t + 1) * P, :])

    for qc in range(NSC):
        qsl = slice(qc * 512, (qc + 1) * 512)
        for t in range(NDT):
            Kt = krot[t]
            for gl in range(2):
                g = 2 * t + gl
                rows = slice(64 * gl, 64 * gl + 64)
                Qz = qrot[2 * t + gl]
                pt = pt_pool.tile([P, NST, 512], dt_in, tag="pt", name=f"pt{g}_{qc}")
                if SCORES_KTP < NST // 2:
                    nc.gpsimd.memset(pt[:, 2 * SCORES_KTP:, :], 0.5)
                for ktp in range(SCORES_KTP if "scores" in PHASES else 0):
                    ps = psum_a.tile([P, 2, 512], F32, tag="ps", name=f"psS{g}{qc}{ktp}")
                    for j in range(2):
                        kt = 2 * ktp + j
                        ksl = slice(kt * P, (kt + 1) * P)
                        nc.tensor.matmul(ps[:, j, :], _mm(Kt[:, ksl]),
                                         _mm(Qz[:, qsl]), start=True, stop=True)
                    # psum fp32 -> bf16 staging copy, then cheap bf16 exp (the
                    # direct fp32-psum exp path measures ~8x slower per element)
                    sa = stg.tile([P, 2, 512], BF16, tag="sa")
                    nc.scalar.mul(sa[:], ps[:], 0.125)
                    nc.scalar.activation(pt[:, 2 * ktp:2 * ktp + 2, :], sa[:],
                                         mybir.ActivationFunctionType.Exp)
                if "attnv" not in PHASES:
                    continue
                ps_o = psum_b.tile([65, 512], F32, tag="po", name=f"po{g}_{qc}")
                for kt in range(ATTNV_KT):
                    nc.tensor.matmul(ps_o[:], _mm(vaug[:, kt, g * 65:(g + 1) * 65]),
                                     _mm(pt[:, kt, :]),
                                     start=(kt == 0), stop=(kt == ATTNV_KT - 1))
                rec = ntmp.tile([1, 512], F32, tag="rec")
                nc.vector.reciprocal(rec[:], ps_o[64:65, :])
                rec_b = ntmp.tile([64, 512], F32, tag="recb")
                nc.gpsimd.partition_broadcast(rec_b[:], rec[:])
                nc.vector.tensor_mul(aall[t][rows, qsl],
                                     ps_o[0:64, :], rec_b[:])
        # output projection for this q-chunk's four s-tiles
        for sti in range(4 if "final" in PHASES else 0):
            st = qc * 4 + sti
            for ec in range(2):
                esl = slice(ec * 512, (ec + 1) * 512)
                ps_f = psum_a.tile([P, 2, 512], F32, tag="ps", name=f"ps_f{st}_{ec}")
                for t in range(NDT):
                    nc.tensor.matmul(ps_f[:, 0, :],
                                     _mm(aall[t][:, st * P:(st + 1) * P]),
                                     _mm(wo_sb[t][:, esl]),
                                     start=(t == 0), stop=(t == NDT - 1))
                osb = opool.tile([P, 512], F32, tag="osb")
                if ACT_OUT:
                    nc.scalar.copy(osb[:], ps_f[:, 0, :])
                else:
                    nc.vector.tensor_copy(osb[:], ps_f[:, 0, :])
                nc.sync.dma_start(out=out[st * P:(st + 1) * P, esl], in_=osb[:])


# ---------------------------------------------------------------------------
# host side
# ---------------------------------------------------------------------------

_PROGRAM = None


def _get_program():
    global _PROGRAM
    if _PROGRAM is None:
        _PROGRAM = build_program()
    return _PROGRAM


def _perm_rows(hh):
    """Row permutation of Wq/Wk for one head-half.

    Per 128-tile t (heads a=2t, b=2t+1): [a evens | b evens | a odds | b odds]
    so RoPE pairs sit at partition distance 64 and each head is two 32-row
    groups at bases {0,64} (head a) / {32,96} (head b).
    """
    base = hh * HPC * D
    rows = []
    for h in range(HPC):
        a = base + h * D
        rows += [a + 2 * i for i in range(32)]
        rows += [a + 2 * i + 1 for i in range(32)]
    return np.array(rows, dtype=np.int64)


def _tables():
    inv_freq = 1.0 / (10000.0 ** (np.arange(0, D, 2, dtype=np.float32) / D))
    freqs = np.arange(S, dtype=np.float32)[:, None] * inv_freq[None, :]  # [S, 32]
    cos = np.cos(freqs).T.astype(np.float32)  # [32, S]
    sin = np.sin(freqs).T.astype(np.float32)
    C = np.tile(cos, (4, 1))  # [128, S]
    Ssig = np.concatenate([-sin, sin, -sin, sin], axis=0).astype(np.float32)
    return np.ascontiguousarray(C), np.ascontiguousarray(Ssig)


def prepare_inputs(query, key, value, Wq, Wk, Wv, Wo, bo):
    dt = _np_dt()
    C, Ssig = _tables()
    xTs = {}
    for b in range(B):
        xTs[b] = tuple(
            np.ascontiguousarray(np.asarray(x[b], np.float32).T).astype(dt)
            for x in (query, key, value)
        )
    per_hh = {}
    for hh in range(2):
        perm = _perm_rows(hh)
        dsl = slice(hh * DPC, (hh + 1) * DPC)
        per_hh[hh] = {
            "wqT": np.ascontiguousarray(np.asarray(Wq, np.float32)[perm, :].T).astype(dt),
            "wkT": np.ascontiguousarray(np.asarray(Wk, np.float32)[perm, :].T).astype(dt),
            "wvT": np.ascontiguousarray(np.asarray(Wv, np.float32)[dsl, :].T).astype(dt),
            "woT": np.ascontiguousarray(np.asarray(Wo, np.float32)[:, dsl].T).astype(dt),
        }
    in_maps = []
    for c in range(8):
        b, hh = c // 2, c % 2
        qTb, kTb, vTb = xTs[b]
        m = {"qT": qTb, "kT": kTb, "vT": vTb, "ctab": C, "stab": Ssig}
        m.update(per_hh[hh])
        in_maps.append(m)
    return in_maps


def kernel(query, key, value, Wq, Wk, Wv, Wo, bo):
    nc = _get_program()
    in_maps = prepare_inputs(query, key, value, Wq, Wk, Wv, Wo, bo)
    res = run_bass_kernel_spmd(nc, in_maps, list(range(8)))
    bo = np.asarray(bo, np.float32)
    out = np.empty((B, S, E), np.float32)
    for b in range(B):
        out[b] = res.results[b * 2]["out"] + res.results[b * 2 + 1]["out"] + bo
    return out


if __name__ == "__main__":
    rng = np.random.default_rng(0)
    ins = {
        "query": rng.standard_normal((B, S, E)).astype(np.float32),
        "key": rng.standard_normal((B, S, E)).astype(np.float32),
        "value": rng.standard_normal((B, S, E)).astype(np.float32),
        "Wq": (rng.standard_normal((E, E)) * 0.02).astype(np.float32),
        "Wk": (rng.standard_normal((E, E)) * 0.02).astype(np.float32),
        "Wv": (rng.standard_normal((E, E)) * 0.02).astype(np.float32),
        "Wo": (rng.standard_normal((E, E)) * 0.02).astype(np.float32),
        "bo": np.zeros((E,), np.float32),
    }
    o = kernel(**ins)
    print("out", o.shape, o.dtype, float(np.abs(o).max()))

